# revision 1
# baseline (speedup 1.0000x reference)
"""MoE FFN with hierarchical KV router — Trainium2 Bass kernel (8 NeuronCores).

Strategy (expert-parallel, per the sharding hint):
  * Host computes the router (l2-norm scores -> softmax over EPB=4 -> top-2 ->
    combine weights) and dispatches tokens by global expert id — the
    control-plane "all-to-all by gid" of the sharding step.
  * All FFN FLOPs run on device. Work is packed into uniform "segments",
    each segment = (W1, b1, W2, b2, CAP gathered tokens, per-token scale):
        out_seg = scale * (relu(x @ W1 + b1) @ W2 + b2)
    - one segment per expert chunk (scale = sigmoid(gate_logit) * combine_w)
    - the shared dense FFN is packed as plain segments too (scale = 1)
    Each of the 8 cores runs G segments (same program, different data).
  * Host un-shards by gathering each token's 3 segment rows (2 expert + 1
    shared) and combining them with the per-token weights:
        y[tok] = gate*w0*row0 + gate*w1*row1 + row_shared

Device kernel: raw Bass (explicit engine streams + semaphores), float32r
matmuls (fp32 rounded to 11 mantissa bits, full-rate PE) with activations kept
transposed ([feature, token]) so both layers use weights as the stationary
operand and no on-device transposes are needed. Per segment, inputs arrive as
four contiguous DMA pieces (A: xt+b1+W1-lo, B: W1-hi, C1: b2+W2-lo, C2: W2-hi)
into NBUF=3 rotating SBUF buffers so all input DMAs issue back-to-back and
matmuls start as soon as piece A lands; mm2 runs in two half-passes over k2 so
only the C2-gated half is exposed after the last DMA byte; outputs stream out
per 128-row group from GpSimd.

Blob layout per segment (per partition p, 4-byte cols):
  A:  [XTO, B1O)  xt  col k*CAP + t   = x[tok t, k*128+p]        (f32r)
      [B1O, W1O)  b1  col m  = b1[m*128+p]                       (f32)
      [W1O, AEND) w1  col m*512 + k*128 + q = W1[k*128+p, m*128+q], m<4
  B:  [AEND,BEND) w1 cols for m>=4                               (f32r)
  C1: [B2O, W2M)  b2  col m2 = b2[m2*128+p] (f32); w2 k2<4       (f32r)
  C2: [W2M, COLS) w2 cols for k2>=4, col k2*C + c = W2[k2*128+p, c]
out[g] = [128, KC*CAP]: col m2*CAP + t = FFN(x)[t, m2*128+p] (unweighted)
"""
import sys

if "/opt/trn_rl_repo" not in sys.path:
    sys.path.insert(0, "/opt/trn_rl_repo")

import numpy as np

N_BUCKET, EPB, TOPK, TAU = 4, 4, 2, 1.0
C, H = 512, 1024
E = N_BUCKET * EPB
KC, KH = C // 128, H // 128  # contraction blocks: 4, 8
N_CORES = 8

_BUILD_CACHE = {}


def _offsets(CAP):
    XTO = 0
    B1O = XTO + KC * CAP
    W1O = B1O + KH            # w1 cols: m*512 + k*128 + q (m-major!)
    AEND = W1O + KC * H // 2  # piece A = [0, AEND): xt, b1, w1 m<4
    BEND = W1O + KC * H       # piece B = [AEND, BEND): w1 m>=4
    B2O = BEND
    W2O = B2O + KC
    W2M = W2O + KH * C // 2   # piece C1 = [B2O, W2M): b2, w2 k2<4
    COLS = W2O + KH * C       # piece C2 = [W2M, COLS): w2 k2>=4
    return XTO, B1O, W1O, W2O, B2O, COLS, AEND, BEND, W2M


def _build_program(G, CAP):
    """Raw-bass program: G segments of CAP tokens through a C->H->C relu FFN."""
    from contextlib import ExitStack

    import concourse.bass as bass
    import concourse.mybir as mybir

    f32 = mybir.dt.float32
    f32r = mybir.dt.float32r
    XTO, B1O, W1O, W2O, B2O, COLS, AEND, BEND, W2M = _offsets(CAP)
    NBUF = min(G, 3)

    nc = bass.Bass("TRN2", target_bir_lowering=False, debug=False)
    blob = nc.declare_dram_parameter("blob", [G, 128, COLS], f32r, isOutput=False)
    out = nc.declare_dram_parameter("out", [G, 128, KC * CAP], f32, isOutput=True)

    def w1col(m, k):
        return W1O + m * 512 + k * 128

    with ExitStack() as ctx:
        BL = [ctx.enter_context(nc.sbuf_tensor(f"bl{i}", [128, COLS], f32r)) for i in range(NBUF)]
        H1 = [ctx.enter_context(nc.sbuf_tensor(f"h1_{i}", [128, KH * CAP], f32r)) for i in range(2)]
        OT = [ctx.enter_context(nc.sbuf_tensor(f"ot{i}", [128, KC * CAP], f32)) for i in range(2)]
        PS = [ctx.enter_context(nc.psum_tensor(f"ps{i}", [128, CAP], f32)) for i in range(8)]
        inA = [ctx.enter_context(nc.semaphore(f"inA{i}")) for i in range(NBUF)]
        inB = [ctx.enter_context(nc.semaphore(f"inB{i}")) for i in range(NBUF)]
        inC1 = [ctx.enter_context(nc.semaphore(f"inC1_{i}")) for i in range(NBUF)]
        inC2 = [ctx.enter_context(nc.semaphore(f"inC2_{i}")) for i in range(NBUF)]
        outS = [ctx.enter_context(nc.semaphore(f"outS{i}")) for i in range(2)]
        pe1 = ctx.enter_context(nc.semaphore("pe1"))
        pe2 = ctx.enter_context(nc.semaphore("pe2"))
        act1 = ctx.enter_context(nc.semaphore("act1"))
        dve1 = ctx.enter_context(nc.semaphore("dve1"))
        block = ctx.enter_context(nc.Block(no_gpsimd_drain=True))

        @block.sync
        def _(sync):
            def issue_blob(g):
                if g >= NBUF:
                    # blob slot g%NBUF recycle: all readers of segment g-NBUF done
                    sync.wait_ge(pe2, 4 * (g - NBUF + 1))
                    sync.wait_ge(act1, 8 * (g - NBUF + 1))
                    sync.wait_ge(dve1, 4 * (g - NBUF + 1))
                bl = BL[g % NBUF][:]
                sl = g % NBUF
                sync.dma_start(out=bl[:, XTO:AEND], in_=blob[g][:, XTO:AEND]).then_inc(inA[sl], 16)
                sync.dma_start(out=bl[:, AEND:BEND], in_=blob[g][:, AEND:BEND]).then_inc(inB[sl], 16)
                sync.dma_start(out=bl[:, B2O:W2M], in_=blob[g][:, B2O:W2M]).then_inc(inC1[sl], 16)
                sync.dma_start(out=bl[:, W2M:COLS], in_=blob[g][:, W2M:COLS]).then_inc(inC2[sl], 16)

            for g in range(G):
                issue_blob(g)
            n_even = (G + 1) // 2
            n_odd = G // 2
            sync.wait_ge(outS[0], 16 * KC * n_even)
            sync.wait_ge(outS[1], 16 * KC * n_odd)

        @block.gpsimd
        def _(gpsimd):
            for g in range(G):
                for m2 in range(KC):
                    gpsimd.wait_ge(dve1, 4 * g + m2 + 1)
                    gpsimd.dma_start(
                        out=out[g][:, m2 * CAP: (m2 + 1) * CAP],
                        in_=OT[g % 2][:, m2 * CAP: (m2 + 1) * CAP],
                    ).then_inc(outS[g % 2], 16)

        @block.tensor
        def _(tensor):
            for g in range(G):
                sl = g % NBUF
                bl = BL[sl][:]
                h1 = H1[g % 2][:]
                # mm1: h1T[m] = W1[:,m]^T @ xT   (accumulate over KC chunks)
                tensor.wait_ge(inA[sl], 16 * (g // NBUF + 1))
                for m in range(KH):
                    if m == 4:
                        tensor.wait_ge(inB[sl], 16 * (g // NBUF + 1))
                    if m >= 4:
                        tensor.wait_ge(act1, 8 * g + (m - 4) + 1)  # ps bank m%4 free
                    for k in range(KC):
                        mm = nc.tensor.matmul(
                            PS[m % 4][:],
                            lhsT=bl[:, w1col(m, k): w1col(m, k) + 128],
                            rhs=bl[:, XTO + k * CAP: XTO + (k + 1) * CAP],
                            start=(k == 0),
                            stop=(k == KC - 1),
                        )
                    mm.then_inc(pe1, 1)
                # mm2: outT[m2] = W2[:,m2]^T @ h1T, two half-passes over k2
                tensor.wait_ge(inC1[sl], 16 * (g // NBUF + 1))
                for m2 in range(KC):
                    if g >= 1:
                        tensor.wait_ge(dve1, 4 * (g - 1) + m2 + 1)  # ps bank 4+m2 free
                    for k2 in range(KH // 2):
                        if m2 == 0:
                            tensor.wait_ge(act1, 8 * g + k2 + 1)  # h1[k2] ready
                        nc.tensor.matmul(
                            PS[4 + m2][:],
                            lhsT=bl[:, W2O + k2 * C + m2 * 128: W2O + k2 * C + (m2 + 1) * 128],
                            rhs=h1[:, k2 * CAP: (k2 + 1) * CAP],
                            start=(k2 == 0),
                            stop=False,
                        )
                tensor.wait_ge(inC2[sl], 16 * (g // NBUF + 1))
                for m2 in range(KC):
                    for k2 in range(KH // 2, KH):
                        if m2 == 0:
                            tensor.wait_ge(act1, 8 * g + k2 + 1)  # h1[k2] ready
                        mm = nc.tensor.matmul(
                            PS[4 + m2][:],
                            lhsT=bl[:, W2O + k2 * C + m2 * 128: W2O + k2 * C + (m2 + 1) * 128],
                            rhs=h1[:, k2 * CAP: (k2 + 1) * CAP],
                            start=False,
                            stop=(k2 == KH - 1),
                        )
                    mm.then_inc(pe2, 1)

        @block.scalar
        def _(scalar):
            for g in range(G):
                bl = BL[g % NBUF][:]
                h1 = H1[g % 2][:]
                for m in range(KH):
                    if g >= 2 and m == 0:
                        scalar.wait_ge(pe2, 4 * (g - 1))  # h1 slot recycle
                    scalar.wait_ge(pe1, 8 * g + m + 1)
                    nc.scalar.activation(
                        h1[:, m * CAP: (m + 1) * CAP],
                        PS[m % 4][:],
                        mybir.ActivationFunctionType.Relu,
                        bias=bl[:, B1O + m: B1O + m + 1].bitcast(f32),
                    ).then_inc(act1, 1)

        @block.vector
        def _(vector):
            for g in range(G):
                bl = BL[g % NBUF][:]
                ot = OT[g % 2][:]
                for m2 in range(KC):
                    if g >= 2 and m2 == 0:
                        vector.wait_ge(outS[g % 2], 16 * KC * (g // 2))  # o_t slot recycle
                    vector.wait_ge(pe2, 4 * g + m2 + 1)
                    nc.vector.tensor_scalar_add(
                        ot[:, m2 * CAP: (m2 + 1) * CAP],
                        PS[4 + m2][:],
                        bl[:, B2O + m2: B2O + m2 + 1].bitcast(f32),
                    ).then_inc(dve1, 1)

    return nc


def _round_f32r(a):
    """Round float32 array to fp32r (round-to-nearest-even at mantissa bit 12)."""
    u = np.ascontiguousarray(a, np.float32).view(np.uint32)
    lsb = (u >> 12) & 1
    r = (u + 0x7FF + lsb) & 0xFFFFF000
    return r.view(np.float32)


def _route(x2, bucket, expert_key):
    """Host router in float64. Returns gid (N,2), combine weights (N,2)."""
    hn = x2 / np.maximum(np.linalg.norm(x2, axis=-1, keepdims=True), 1e-12)
    keys = expert_key / np.maximum(
        np.linalg.norm(expert_key, axis=-1, keepdims=True), 1e-12
    )
    kb = keys[bucket]  # (N, EPB, C)
    score = np.einsum("nc,nec->ne", hn, kb) / max(TAU, 1e-6)
    score -= score.max(axis=-1, keepdims=True)
    p = np.exp(score)
    p /= p.sum(axis=-1, keepdims=True)
    local = np.argsort(-p, axis=-1, kind="stable")[:, :TOPK]  # (N, 2)
    topv = np.take_along_axis(p, local, axis=-1)
    w = topv / (topv.sum(axis=-1, keepdims=True) + 1e-9)
    gid = bucket[:, None] * EPB + local
    return gid, w


def kernel(**inputs):
    from concourse.bass_utils import run_bass_kernel_spmd

    x = np.asarray(inputs["x"], dtype=np.float32)
    op_id = np.asarray(inputs["op_id"]).astype(np.int64)
    expert_key = np.asarray(inputs["expert_key"], dtype=np.float64)
    sW1 = np.asarray(inputs["sW1"], dtype=np.float32)
    sb1 = np.asarray(inputs["sb1"], dtype=np.float32)
    sW2 = np.asarray(inputs["sW2"], dtype=np.float32)
    sb2 = np.asarray(inputs["sb2"], dtype=np.float32)
    eW1 = np.asarray(inputs["eW1"], dtype=np.float32)
    eb1 = np.asarray(inputs["eb1"], dtype=np.float32)
    eW2 = np.asarray(inputs["eW2"], dtype=np.float32)
    eb2 = np.asarray(inputs["eb2"], dtype=np.float32)
    gate_logit = float(np.asarray(inputs["gate_logit"]))

    B, T, Cc = x.shape
    assert Cc == C
    N = B * T
    x2 = x.reshape(N, C)
    bucket = np.clip(op_id.reshape(-1), 0, N_BUCKET - 1)

    gid, w = _route(x2.astype(np.float64), bucket, expert_key)
    gate = 1.0 / (1.0 + np.exp(-gate_logit))

    # ---- pack work into segments of CAP token slots --------------------
    flat_gid = gid.reshape(-1)  # (N*2,) ; slot i -> token i//2
    sorted_slots = np.argsort(flat_gid, kind="stable")
    counts = np.bincount(flat_gid, minlength=E)

    # choose CAP: minimize G = ceil(S/8), then CAP
    best = None
    for cap in range(256, 513, 32):
        S = int(sum(-(-c // cap) for c in counts if c > 0)) + -(-N // cap)
        Gc = -(-S // N_CORES)
        key = (Gc, cap)
        if best is None or key < best[:2]:
            best = (Gc, cap, S)
    G, CAP, S = best
    S_pad = G * N_CORES
    XTO, B1O, W1O, W2O, B2O, COLS, AEND, BEND, W2M = _offsets(CAP)

    blob = np.zeros((S_pad, 128, COLS), np.float32)
    slot_flat = np.zeros((3, N), np.int64)  # each token: 2 expert rows + 1 shared row
    x2T_r = _round_f32r(x2.T)  # (C, N)

    def fill_segment(s, w1_, b1_, w2_, b2_, tok_idx):
        n = len(tok_idx)
        # w1 m-major: col m*512 + k*128 + q = W1[k*128+p, m*128+q]
        w1m = w1_.reshape(KC, 128, KH, 128).transpose(1, 2, 0, 3).reshape(128, KC * H)
        blob[s, :, W1O:BEND] = w1m
        blob[s, :, W2O:COLS] = w2_.reshape(KH, 128, C).transpose(1, 0, 2).reshape(128, KH * C)
        xg = x2T_r[:, tok_idx]  # (C, n)
        blob[s, :, XTO:B1O].reshape(128, KC, CAP)[:, :, :n] = (
            xg.reshape(KC, 128, n).transpose(1, 0, 2)
        )
        blob[s, :, B1O:W1O] = b1_.reshape(KH, 128).T
        blob[s, :, B2O:W2O] = b2_.reshape(KC, 128).T

    ew1r = _round_f32r(eW1)
    ew2r = _round_f32r(eW2)
    sw1r = _round_f32r(sW1)
    sw2r = _round_f32r(sW2)

    s = 0
    pos = 0
    for e in range(E):
        cnt = int(counts[e])
        slots_e = sorted_slots[pos: pos + cnt]
        pos += cnt
        for lo in range(0, cnt, CAP):
            chunk = slots_e[lo: lo + CAP]
            toks = chunk // TOPK
            fill_segment(s, ew1r[e], eb1[e], ew2r[e], eb2[e], toks)
            slot_flat[chunk % TOPK, toks] = s * CAP + np.arange(len(chunk))
            s += 1
    for lo in range(0, N, CAP):
        toks = np.arange(lo, min(lo + CAP, N))
        fill_segment(s, sw1r, sb1, sw2r, sb2, toks)
        slot_flat[2, toks] = s * CAP + np.arange(len(toks))
        s += 1
    assert s == S <= S_pad

    # ---- compile + run on the 8 cores ----------------------------------
    key = (G, CAP)
    if key not in _BUILD_CACHE:
        _BUILD_CACHE[key] = _build_program(G, CAP)
    nc = _BUILD_CACHE[key]

    in_maps = [{"blob": blob[c * G: (c + 1) * G]} for c in range(N_CORES)]

    import os

    trace = bool(os.environ.get("BASS_TRACE"))
    res = run_bass_kernel_spmd(
        nc,
        in_maps,
        core_ids=list(range(N_CORES)),
        trace=trace,
        trace_cores=list(range(N_CORES)) if trace else None,
    )
    global LAST_EXEC_NS, LAST_RESULTS
    LAST_EXEC_NS = res.exec_time_ns
    LAST_RESULTS = res

    # ---- un-shard: gather each token's 3 rows and add ------------------
    # core output (G, 128, KC*CAP): col m2*CAP+t, C index = m2*128+p
    allout = np.empty((S_pad * CAP, C), np.float32)
    for c in range(N_CORES):
        o = np.asarray(res.results[c]["out"]).reshape(G, 128, KC, CAP)
        o = o.transpose(0, 3, 2, 1).reshape(G * CAP, C)  # token-major
        allout[c * G * CAP: (c + 1) * G * CAP] = o

    wf = (gate * w).astype(np.float32)  # (N, 2) combine weights
    y = (
        allout[slot_flat[0]] * wf[:, 0:1]
        + allout[slot_flat[1]] * wf[:, 1:2]
        + allout[slot_flat[2]]
    )
    return y.reshape(B, T, C)


LAST_EXEC_NS = None
LAST_RESULTS = None



# revision 5
# speedup vs baseline: 1.3757x; 1.3757x over previous
"""MoE FFN with hierarchical KV router — Trainium2 Bass kernel (8 NeuronCores).

Expert-parallel, weights-resident design:
  * Host computes the router (l2-norm scores -> softmax over EPB=4 -> top-2 ->
    combine weights) and dispatches tokens by global expert id.
  * Each core owns TWO experts (big+small pairing over the 16 experts) plus a
    replica of the shared FFN serving 2048/8 = 256 tokens. All weights are
    loaded into SBUF exactly once per core (they stay resident), tokens are
    streamed through three fixed-size segments [CA | CB | CS]:
        seg 0: expert A  (CA token slots)   seg 1: expert B (CB slots)
        seg 2: shared FFN (CS = 256 slots)
    out_seg = relu(x @ W1 + b1) @ W2 + b2   (unweighted; host combines)
  * Precision: weights/activations in bf16. When the expert path is strongly
    attenuated (sigmoid(gate_logit) <= 0.25) the expert segments run in
    fp8-e4m3 with power-of-2 scaling and DoubleRow matmuls (2x PE rate).
    Shared FFN always bf16. Outputs bf16, combined in fp32 on host.
  * Activations travel transposed ([feature, token]) so weights are the
    stationary matmul operand; no on-device transposes.

Per-core traffic ~5.5-8 MB (vs ~14.5 MB for the segment-blob design), PE work
~34-52k cycles; both sides land around 17-22 us.
"""
import sys

if "/opt/trn_rl_repo" not in sys.path:
    sys.path.insert(0, "/opt/trn_rl_repo")

import numpy as np

N_BUCKET, EPB, TOPK, TAU = 4, 4, 2, 1.0
C, H = 512, 1024
E = N_BUCKET * EPB
KC, KH = C // 128, H // 128  # contraction blocks: 4, 8
N_CORES = 8
PSUM_CAP = 512
BCOLS = KH + KC  # bias cols per segment

_BUILD_CACHE = {}


def _ensure_ntff_hook():
    """Polyfill antenv.axon_hooks (absent in some agent images) so
    run_bass_kernel_spmd(trace=True) can fetch NTFF profiles."""
    try:
        from antenv.axon_hooks import get_axon_ntff_profile_hook  # noqa: F401
        return
    except ImportError:
        pass
    import types

    try:
        import antenv
        from trn_agent_boot.trn_boot import _ntff_profile_via_ctypes

        hook = _ntff_profile_via_ctypes("/opt/axon/libaxon_pjrt.so")
        mod = types.ModuleType("antenv.axon_hooks")
        state = {"hook": hook}
        mod.get_axon_ntff_profile_hook = lambda: state["hook"]
        mod.set_axon_ntff_profile_hook = lambda h: state.update(hook=h)
        sys.modules["antenv.axon_hooks"] = mod
        antenv.axon_hooks = mod
    except Exception:
        pass


def _build_program(CA, CB, CS, fp8, sc1e, sc2e):
    """One-shot program: 3 segments (expert A, expert B, shared) per core.

    All inputs DMA once into SBUF; segments run back-to-back on the PE with
    scalar(relu+b1) and vector(+b2) trailing; outputs DMA from gpsimd.
    """
    from contextlib import ExitStack

    import concourse.bass as bass
    import concourse.mybir as mybir

    f32 = mybir.dt.float32
    bf16 = mybir.dt.bfloat16
    dt_e = mybir.dt.float8e4 if fp8 else bf16
    CAB = CA + CB
    OC = KC * (CAB + CS)
    estep = 2 if fp8 else 1
    pm = mybir.MatmulPerfMode.DoubleRow if fp8 else None

    nc = bass.Bass("TRN2", target_bir_lowering=False, debug=False)
    e1d = nc.declare_dram_parameter("e1", [2, 128, KC, H], dt_e, isOutput=False)
    e2d = nc.declare_dram_parameter("e2", [2, 128, KH, C], dt_e, isOutput=False)
    s1d = nc.declare_dram_parameter("s1", [128, KC, H], bf16, isOutput=False)
    s2d = nc.declare_dram_parameter("s2", [128, KH, C], bf16, isOutput=False)
    xted = nc.declare_dram_parameter("xte", [128, KC, CAB], dt_e, isOutput=False)
    xtsd = nc.declare_dram_parameter("xts", [128, KC, CS], bf16, isOutput=False)
    biasd = nc.declare_dram_parameter("bias", [128, 3 * BCOLS], f32, isOutput=False)
    outd = nc.declare_dram_parameter("out", [128, OC], bf16, isOutput=True)

    with ExitStack() as ctx:
        E1 = ctx.enter_context(nc.sbuf_tensor("E1", [128, 2 * KC, H], dt_e))
        E2 = ctx.enter_context(nc.sbuf_tensor("E2", [128, 2 * KH, C], dt_e))
        S1 = ctx.enter_context(nc.sbuf_tensor("S1", [128, KC, H], bf16))
        S2 = ctx.enter_context(nc.sbuf_tensor("S2", [128, KH, C], bf16))
        XE = ctx.enter_context(nc.sbuf_tensor("XE", [128, KC, CAB], dt_e))
        XS = ctx.enter_context(nc.sbuf_tensor("XS", [128, KC, CS], bf16))
        BI = ctx.enter_context(nc.sbuf_tensor("BI", [128, 3 * BCOLS], f32))
        H1A = ctx.enter_context(nc.sbuf_tensor("H1A", [128, KH, CA], dt_e))
        H1B = ctx.enter_context(nc.sbuf_tensor("H1B", [128, KH, CB], dt_e))
        H1S = ctx.enter_context(nc.sbuf_tensor("H1S", [128, KH, CS], bf16))
        OT = ctx.enter_context(nc.sbuf_tensor("OT", [128, OC], bf16))
        PS = [ctx.enter_context(nc.psum_tensor(f"ps{i}", [128, PSUM_CAP], f32)) for i in range(8)]

        inB = ctx.enter_context(nc.semaphore("inB"))
        inXE = ctx.enter_context(nc.semaphore("inXE"))
        inE1 = [ctx.enter_context(nc.semaphore(f"inE1_{j}")) for j in range(2)]
        inE2 = [ctx.enter_context(nc.semaphore(f"inE2_{j}")) for j in range(2)]
        inXS = ctx.enter_context(nc.semaphore("inXS"))
        inS1 = ctx.enter_context(nc.semaphore("inS1"))
        inS2 = ctx.enter_context(nc.semaphore("inS2"))
        pe1 = ctx.enter_context(nc.semaphore("pe1"))
        pe2 = ctx.enter_context(nc.semaphore("pe2"))
        act1 = ctx.enter_context(nc.semaphore("act1"))
        dve1 = ctx.enter_context(nc.semaphore("dve1"))
        outS = ctx.enter_context(nc.semaphore("outS"))
        block = ctx.enter_context(nc.Block(no_gpsimd_drain=True))

        # segment descriptors: (cap, h1, mm1 weights fn, mm2 weights fn,
        #   rhs fn, step, perf_mode, out col offset, sc1, sc2, mm1 wait sem,
        #   mm2 wait sem)
        def ew1(j):
            return lambda kk, st, m: E1[:][:, j * KC + kk: j * KC + kk + st, m * 128:(m + 1) * 128]

        def ew2(j):
            return lambda kk, st, m2: E2[:][:, j * KH + kk: j * KH + kk + st, m2 * 128:(m2 + 1) * 128]

        def sw1(kk, st, m):
            return S1[:][:, kk: kk + st, m * 128:(m + 1) * 128]

        def sw2(kk, st, m2):
            return S2[:][:, kk: kk + st, m2 * 128:(m2 + 1) * 128]

        def exA(kk, st):
            return XE[:][:, kk: kk + st, 0:CA]

        def exB(kk, st):
            return XE[:][:, kk: kk + st, CA:CAB]

        def exS(kk, st):
            return XS[:][:, kk: kk + st, 0:CS]

        segs = [
            dict(cap=CA, h1=H1A, w1=ew1(0), w2=ew2(0), x=exA, step=estep, pm=pm,
                 ooff=0, sc1=sc1e, sc2=sc2e, mw1=inE1[0], mw2=inE2[0]),
            dict(cap=CB, h1=H1B, w1=ew1(1), w2=ew2(1), x=exB, step=estep, pm=pm,
                 ooff=KC * CA, sc1=sc1e, sc2=sc2e, mw1=inE1[1], mw2=inE2[1]),
            dict(cap=CS, h1=H1S, w1=sw1, w2=sw2, x=exS, step=1, pm=None,
                 ooff=KC * CAB, sc1=1.0, sc2=1.0, mw1=inS1, mw2=inS2),
        ]

        @block.sync
        def _(sync):
            sync.dma_start(out=BI[:], in_=biasd[:]).then_inc(inB, 16)
            sync.dma_start(out=XE[:], in_=xted[:]).then_inc(inXE, 16)
            sync.dma_start(out=E1[:][:, 0:KC, :], in_=e1d[0]).then_inc(inE1[0], 16)
            sync.dma_start(out=E2[:][:, 0:KH, :], in_=e2d[0]).then_inc(inE2[0], 16)
            sync.dma_start(out=E1[:][:, KC:2 * KC, :], in_=e1d[1]).then_inc(inE1[1], 16)
            sync.dma_start(out=E2[:][:, KH:2 * KH, :], in_=e2d[1]).then_inc(inE2[1], 16)
            sync.dma_start(out=XS[:], in_=xtsd[:]).then_inc(inXS, 16)
            sync.dma_start(out=S1[:], in_=s1d[:]).then_inc(inS1, 16)
            sync.dma_start(out=S2[:], in_=s2d[:]).then_inc(inS2, 16)
            sync.wait_ge(outS, 16 * 6)

        @block.tensor
        def _(tensor):
            tensor.wait_ge(inXE, 16)
            for s, sg in enumerate(segs):
                cap, step = sg["cap"], sg["step"]
                if s == 2:
                    tensor.wait_ge(inXS, 16)
                tensor.wait_ge(sg["mw1"], 16)
                # mm1: h1[m] = sum_k W1[k,m]^T @ xT[k]
                for m in range(KH):
                    # psum bank m%4 recycle: previous relu reader done
                    if m >= 4:
                        tensor.wait_ge(act1, 8 * s + (m - 4) + 1)
                    elif s > 0:
                        tensor.wait_ge(act1, 8 * (s - 1) + (m + 4) + 1)
                    for kk in range(0, KC, step):
                        mm = nc.tensor.matmul(
                            PS[m % 4][:, :cap],
                            lhsT=sg["w1"](kk, step, m),
                            rhs=sg["x"](kk, step),
                            start=(kk == 0),
                            stop=(kk + step >= KC),
                            perf_mode=sg["pm"],
                        )
                    mm.then_inc(pe1, 1)
                # mm2: out[m2] = sum_k2 W2[k2,m2]^T @ h1[k2]
                tensor.wait_ge(sg["mw2"], 16)
                h1 = sg["h1"][:]
                for m2 in range(KC):
                    if s > 0:
                        tensor.wait_ge(dve1, 4 * (s - 1) + m2 + 1)
                    for kk in range(0, KH, step):
                        if m2 == 0:
                            tensor.wait_ge(act1, 8 * s + kk + step)
                        mm = nc.tensor.matmul(
                            PS[4 + m2][:, :cap],
                            lhsT=sg["w2"](kk, step, m2),
                            rhs=h1[:, kk: kk + step, 0:cap],
                            start=(kk == 0),
                            stop=(kk + step >= KH),
                            perf_mode=sg["pm"],
                        )
                    mm.then_inc(pe2, 1)

        @block.scalar
        def _(scalar):
            import concourse.mybir as mybir_

            scalar.wait_ge(inB, 16)
            for s, sg in enumerate(segs):
                cap = sg["cap"]
                h1 = sg["h1"][:]
                for m in range(KH):
                    scalar.wait_ge(pe1, 8 * s + m + 1)
                    nc.scalar.activation(
                        h1[:, m, 0:cap],
                        PS[m % 4][:, :cap],
                        mybir_.ActivationFunctionType.Relu,
                        bias=BI[:][:, s * BCOLS + m: s * BCOLS + m + 1],
                        scale=float(sg["sc1"]),
                    ).then_inc(act1, 1)

        @block.vector
        def _(vector):
            import concourse.mybir as mybir_

            vector.wait_ge(inB, 16)
            for s, sg in enumerate(segs):
                cap = sg["cap"]
                for m2 in range(KC):
                    vector.wait_ge(pe2, 4 * s + m2 + 1)
                    nc.vector.tensor_scalar(
                        OT[:][:, sg["ooff"] + m2 * cap: sg["ooff"] + (m2 + 1) * cap],
                        PS[4 + m2][:, :cap],
                        float(sg["sc2"]),
                        BI[:][:, s * BCOLS + KH + m2: s * BCOLS + KH + m2 + 1],
                        op0=mybir_.AluOpType.mult,
                        op1=mybir_.AluOpType.add,
                    ).then_inc(dve1, 1)

        @block.gpsimd
        def _(gpsimd):
            # seg 0/1: one DMA each; seg 2 split per m2 group for a short tail
            for s, sg in enumerate(segs[:2]):
                gpsimd.wait_ge(dve1, 4 * s + 4)
                gpsimd.dma_start(
                    out=outd[:, sg["ooff"]: sg["ooff"] + KC * sg["cap"]],
                    in_=OT[:][:, sg["ooff"]: sg["ooff"] + KC * sg["cap"]],
                ).then_inc(outS, 16)
            sg = segs[2]
            for m2 in range(KC):
                gpsimd.wait_ge(dve1, 8 + m2 + 1)
                gpsimd.dma_start(
                    out=outd[:, sg["ooff"] + m2 * CS: sg["ooff"] + (m2 + 1) * CS],
                    in_=OT[:][:, sg["ooff"] + m2 * CS: sg["ooff"] + (m2 + 1) * CS],
                ).then_inc(outS, 16)

    return nc


def _run_coresim(CA, CB, CS, fp8, sc1e, sc2e, in_maps):
    """Local CoreSim execution (numerics check without hardware)."""
    from types import SimpleNamespace

    from concourse.bass_interp import CoreSim

    results = []
    for c, im in enumerate(in_maps):
        nc = _build_program(CA, CB, CS, fp8, sc1e, sc2e)
        if not nc.is_finalized():
            nc.finalize()
        sim = CoreSim(nc, core_id=0, publish_trace=False)
        for name, val in im.items():
            sim.tensor(name)[:] = val
        sim.simulate()
        results.append({"out": np.array(sim.tensor("out"))})
        print(f"  coresim core {c} done", flush=True)
    return SimpleNamespace(results=results, exec_time_ns=None)


def _route(x2, bucket, expert_key):
    """Host router in float64. Returns gid (N,2), combine weights (N,2)."""
    hn = x2 / np.maximum(np.linalg.norm(x2, axis=-1, keepdims=True), 1e-12)
    keys = expert_key / np.maximum(
        np.linalg.norm(expert_key, axis=-1, keepdims=True), 1e-12
    )
    kb = keys[bucket]  # (N, EPB, C)
    score = np.einsum("nc,nec->ne", hn, kb) / max(TAU, 1e-6)
    score -= score.max(axis=-1, keepdims=True)
    p = np.exp(score)
    p /= p.sum(axis=-1, keepdims=True)
    local = np.argsort(-p, axis=-1, kind="stable")[:, :TOPK]  # (N, 2)
    topv = np.take_along_axis(p, local, axis=-1)
    w = topv / (topv.sum(axis=-1, keepdims=True) + 1e-9)
    gid = bucket[:, None] * EPB + local
    return gid, w


def _pow2floor(v):
    return float(2.0 ** np.floor(np.log2(max(v, 1e-30))))


def _ceil16(n):
    return max(16, -(-int(n) // 16) * 16)


def _wpack1(w1):  # (C,H) -> [128, KC, H], block k = rows k*128..k*128+127
    return np.ascontiguousarray(w1.reshape(KC, 128, H).transpose(1, 0, 2))


def _wpack2(w2):  # (H,C) -> [128, KH, C]
    return np.ascontiguousarray(w2.reshape(KH, 128, C).transpose(1, 0, 2))


def kernel(**inputs):
    import ml_dtypes

    _ensure_ntff_hook()
    from concourse.bass_utils import run_bass_kernel_spmd

    bf16 = ml_dtypes.bfloat16
    f8 = ml_dtypes.float8_e4m3

    x = np.asarray(inputs["x"], dtype=np.float32)
    op_id = np.asarray(inputs["op_id"]).astype(np.int64)
    expert_key = np.asarray(inputs["expert_key"], dtype=np.float64)
    sW1 = np.asarray(inputs["sW1"], dtype=np.float32)
    sb1 = np.asarray(inputs["sb1"], dtype=np.float32)
    sW2 = np.asarray(inputs["sW2"], dtype=np.float32)
    sb2 = np.asarray(inputs["sb2"], dtype=np.float32)
    eW1 = np.asarray(inputs["eW1"], dtype=np.float32)
    eb1 = np.asarray(inputs["eb1"], dtype=np.float32)
    eW2 = np.asarray(inputs["eW2"], dtype=np.float32)
    eb2 = np.asarray(inputs["eb2"], dtype=np.float32)
    gate_logit = float(np.asarray(inputs["gate_logit"]))

    B, T, Cc = x.shape
    assert Cc == C
    N = B * T
    assert N % N_CORES == 0
    x2 = x.reshape(N, C)
    bucket = np.clip(op_id.reshape(-1), 0, N_BUCKET - 1)

    gid, w = _route(x2.astype(np.float64), bucket, expert_key)
    gate = 1.0 / (1.0 + np.exp(-gate_logit))

    # ---- assign experts to cores: big+small pairing ---------------------
    flat_gid = gid.reshape(-1)  # slot i -> token i//2
    sorted_slots = np.argsort(flat_gid, kind="stable")
    counts = np.bincount(flat_gid, minlength=E)
    starts = np.concatenate([[0], np.cumsum(counts)])
    assert counts.max() <= PSUM_CAP, "expert overflow; need chunked fallback"
    order = np.argsort(-counts, kind="stable")
    CA = _ceil16(counts[order[0]])
    CB = _ceil16(counts[order[8]])
    CS = N // N_CORES
    CAB, TOT = CA + CB, CA + CB + CS
    OC = KC * TOT

    fp8 = gate <= 0.25
    if fp8:
        s_x = _pow2floor(192.0 / max(np.abs(x2).max(), 1e-6))
        s_w1 = _pow2floor(192.0 / max(np.abs(eW1).max(), 1e-6))
        s_w2 = _pow2floor(192.0 / max(np.abs(eW2).max(), 1e-6))
        xn = np.linalg.norm(x2, axis=1).max()
        w1n = np.linalg.norm(eW1, axis=1).max()
        h1_bound = xn * w1n + np.abs(eb1).max() + 1e-6
        s_h = _pow2floor(192.0 / h1_bound)
        sc1e = s_h / (s_x * s_w1)
        sc2e = 1.0 / (s_h * s_w2)
    else:
        s_x = s_w1 = s_w2 = s_h = 1.0
        sc1e, sc2e = 1.0, 1.0
    dt_e = f8 if fp8 else bf16

    key = (CA, CB, CS, fp8, sc1e, sc2e)
    if key not in _BUILD_CACHE:
        _BUILD_CACHE[key] = _build_program(CA, CB, CS, fp8, sc1e, sc2e)
    nc = _BUILD_CACHE[key]

    # ---- host packing ---------------------------------------------------
    x2T = np.ascontiguousarray(x2.T)  # (C, N)
    s1_host = np.ascontiguousarray(_wpack1(sW1).astype(bf16))
    s2_host = np.ascontiguousarray(_wpack2(sW2).astype(bf16))

    slot_flat = np.zeros((3, N), np.int64)
    in_maps = []
    for c in range(N_CORES):
        eA, eB = int(order[c]), int(order[15 - c])
        e1h = np.zeros((2, 128, KC, H), dt_e)
        e2h = np.zeros((2, 128, KH, C), dt_e)
        e1h[0] = (_wpack1(eW1[eA]) * s_w1).astype(dt_e)
        e1h[1] = (_wpack1(eW1[eB]) * s_w1).astype(dt_e)
        e2h[0] = (_wpack2(eW2[eA]) * s_w2).astype(dt_e)
        e2h[1] = (_wpack2(eW2[eB]) * s_w2).astype(dt_e)

        xte = np.zeros((128, KC, CAB), dt_e)
        biash = np.zeros((128, 3 * BCOLS), np.float32)
        for j, (e, off, capj) in enumerate([(eA, 0, CA), (eB, CA, CB)]):
            toks = (sorted_slots[starts[e]: starts[e + 1]] // TOPK).astype(np.int64)
            n = len(toks)
            xg = x2T[:, toks] * s_x  # (C, n)
            xte[:, :, off: off + n] = xg.reshape(KC, 128, n).transpose(1, 0, 2).astype(dt_e)
            biash[:, j * BCOLS: j * BCOLS + KH] = eb1[e].reshape(KH, 128).T * s_h
            biash[:, j * BCOLS + KH: (j + 1) * BCOLS] = eb2[e].reshape(KC, 128).T
            chunk = sorted_slots[starts[e]: starts[e + 1]]
            slot_flat[chunk % TOPK, toks] = c * TOT + off + np.arange(n)
        stoks = np.arange(c * CS, (c + 1) * CS)
        xts = np.ascontiguousarray(
            x2T[:, stoks].reshape(KC, 128, CS).transpose(1, 0, 2)
        ).astype(bf16)
        biash[:, 2 * BCOLS: 2 * BCOLS + KH] = sb1.reshape(KH, 128).T
        biash[:, 2 * BCOLS + KH: 3 * BCOLS] = sb2.reshape(KC, 128).T
        slot_flat[2, stoks] = c * TOT + CAB + np.arange(CS)

        in_maps.append({
            "e1": e1h, "e2": e2h, "s1": s1_host, "s2": s2_host,
            "xte": xte, "xts": xts, "bias": biash,
        })

    # ---- run on the 8 cores --------------------------------------------
    import os

    global LAST_EXEC_NS, LAST_RESULTS
    if os.environ.get("BASS_SIM"):
        res = _run_coresim(CA, CB, CS, fp8, sc1e, sc2e, in_maps)
    else:
        trace = bool(os.environ.get("BASS_TRACE"))
        res = run_bass_kernel_spmd(
            nc,
            in_maps,
            core_ids=list(range(N_CORES)),
            trace=trace,
            trace_cores=list(range(N_CORES)) if trace else None,
        )
        LAST_EXEC_NS = res.exec_time_ns
        LAST_RESULTS = res

    # ---- un-shard: gather each token's 3 rows and combine ---------------
    allout = np.empty((N_CORES * TOT, C), np.float32)
    caps = [(0, CA), (KC * CA, CB), (KC * CAB, CS)]
    for c in range(N_CORES):
        o = np.asarray(res.results[c]["out"]).astype(np.float32)  # [128, OC]
        row0 = c * TOT
        for ooff, cap in caps:
            blk = o[:, ooff: ooff + KC * cap].reshape(128, KC, cap)
            allout[row0: row0 + cap] = blk.transpose(2, 1, 0).reshape(cap, C)
            row0 += cap

    wf = (gate * w).astype(np.float32)  # (N, 2)
    y = (
        allout[slot_flat[0]] * wf[:, 0:1]
        + allout[slot_flat[1]] * wf[:, 1:2]
        + allout[slot_flat[2]]
    )
    return y.reshape(B, T, C).astype(np.float32)


LAST_EXEC_NS = None
LAST_RESULTS = None


# revision 6
# speedup vs baseline: 1.4634x; 1.0637x over previous
"""MoE FFN with hierarchical KV router — Trainium2 Bass kernel (8 NeuronCores).

Expert-parallel, weights-resident design:
  * Host computes the router (l2-norm scores -> softmax over EPB=4 -> top-2 ->
    combine weights) and dispatches tokens by global expert id.
  * Each core owns TWO experts (big+small pairing over the 16 experts) plus a
    replica of the shared FFN serving 2048/8 = 256 tokens. All weights are
    loaded into SBUF exactly once per core (they stay resident); tokens run
    through three fixed-size segments [CA | CB | CS]:
        seg 0: expert A  (CA token slots)   seg 1: expert B (CB slots)
        seg 2: shared FFN (CS = 256 slots)
    out_seg = relu(x @ W1 + b1) @ W2 + b2   (unweighted; host combines)
  * Precision: bf16 everywhere; when the expert path is strongly attenuated
    (sigmoid(gate_logit) <= 0.25) the expert segments use fp8-e4m3 inputs
    with power-of-2 scaling. Outputs bf16, combined in fp32 on host.
  * Activations travel transposed ([feature, token]) so weights are the
    stationary matmul operand; no on-device transposes.

Schedule notes (from NTFF profiling):
  * HWDGE dma_start costs ~0.7-1us of issue time on the issuing engine, so
    input DMAs are need-ordered on the sync ring (first expert's W1 m<4
    half + its tokens first) and the bias ride the scalar ring.
  * W1 is packed m-major / W2 m2-major so the PE can start after the first
    W1 piece instead of the whole tile.
  * The scalar engine runs a dummy relu right after the bias lands to pull
    the lazy ACT_TABLE_LOAD (~1.5us) off the first real relu.
  * Outputs go out per (segment, m2-group) on the sync HWDGE ring (the
    gpsimd SWDGE path measured ~55 GB/s and added ~4us of tail).
"""
import sys

if "/opt/trn_rl_repo" not in sys.path:
    sys.path.insert(0, "/opt/trn_rl_repo")

import numpy as np

N_BUCKET, EPB, TOPK, TAU = 4, 4, 2, 1.0
C, H = 512, 1024
E = N_BUCKET * EPB
KC, KH = C // 128, H // 128  # contraction blocks: 4, 8
N_CORES = 8
PSUM_CAP = 512
BCOLS = KH + KC  # bias cols per segment

_BUILD_CACHE = {}


def _ensure_ntff_hook():
    """Polyfill antenv.axon_hooks (absent in some agent images) so
    run_bass_kernel_spmd(trace=True) can fetch NTFF profiles."""
    try:
        from antenv.axon_hooks import get_axon_ntff_profile_hook  # noqa: F401
        return
    except ImportError:
        pass
    import types

    try:
        import antenv
        from trn_agent_boot.trn_boot import _ntff_profile_via_ctypes

        hook = _ntff_profile_via_ctypes("/opt/axon/libaxon_pjrt.so")
        mod = types.ModuleType("antenv.axon_hooks")
        state = {"hook": hook}
        mod.get_axon_ntff_profile_hook = lambda: state["hook"]
        mod.set_axon_ntff_profile_hook = lambda h: state.update(hook=h)
        sys.modules["antenv.axon_hooks"] = mod
        antenv.axon_hooks = mod
    except Exception:
        pass


def _build_program(CA, CB, CS, fp8, sc1e, sc2e):
    """One-shot program: 3 segments (expert A, expert B, shared) per core."""
    from contextlib import ExitStack

    import concourse.bass as bass
    import concourse.mybir as mybir

    f32 = mybir.dt.float32
    bf16 = mybir.dt.bfloat16
    dt_e = mybir.dt.float8e4 if fp8 else bf16
    OC = KC * (CA + CB + CS)

    nc = bass.Bass("TRN2", target_bir_lowering=False, debug=False)
    # W1 m-major: e1[j, half, p, m', k*128+q] = W1[k*128+p, (half*4+m')*128+q]
    e1d = nc.declare_dram_parameter("e1", [2, 2, 128, KH // 2, KC * 128], dt_e, isOutput=False)
    # W2 m2-major: e2[j, p, m2, k2*128+c'] = W2[k2*128+p, m2*128+c']
    e2d = nc.declare_dram_parameter("e2", [2, 128, KC, KH * 128], dt_e, isOutput=False)
    s1d = nc.declare_dram_parameter("s1", [128, KH, KC * 128], bf16, isOutput=False)
    s2d = nc.declare_dram_parameter("s2", [128, KC, KH * 128], bf16, isOutput=False)
    xad = nc.declare_dram_parameter("xa", [128, KC, CA], dt_e, isOutput=False)
    xbd = nc.declare_dram_parameter("xb", [128, KC, CB], dt_e, isOutput=False)
    xsd = nc.declare_dram_parameter("xs", [128, KC, CS], bf16, isOutput=False)
    biasd = nc.declare_dram_parameter("bias", [128, 3 * BCOLS], f32, isOutput=False)
    outd = nc.declare_dram_parameter("out", [128, OC], bf16, isOutput=True)

    with ExitStack() as ctx:
        E1 = ctx.enter_context(nc.sbuf_tensor("E1", [128, 2 * KH, KC * 128], dt_e))
        E2 = ctx.enter_context(nc.sbuf_tensor("E2", [128, 2 * KC, KH * 128], dt_e))
        S1 = ctx.enter_context(nc.sbuf_tensor("S1", [128, KH, KC * 128], bf16))
        S2 = ctx.enter_context(nc.sbuf_tensor("S2", [128, KC, KH * 128], bf16))
        XA = ctx.enter_context(nc.sbuf_tensor("XA", [128, KC, CA], dt_e))
        XB = ctx.enter_context(nc.sbuf_tensor("XB", [128, KC, CB], dt_e))
        XS = ctx.enter_context(nc.sbuf_tensor("XS", [128, KC, CS], bf16))
        BI = ctx.enter_context(nc.sbuf_tensor("BI", [128, 3 * BCOLS], f32))
        SC = ctx.enter_context(nc.sbuf_tensor("SC", [128, 1], f32))
        H1A = ctx.enter_context(nc.sbuf_tensor("H1A", [128, KH, CA], dt_e))
        H1B = ctx.enter_context(nc.sbuf_tensor("H1B", [128, KH, CB], dt_e))
        H1S = ctx.enter_context(nc.sbuf_tensor("H1S", [128, KH, CS], bf16))
        OT = ctx.enter_context(nc.sbuf_tensor("OT", [128, OC], bf16))
        PS = [ctx.enter_context(nc.psum_tensor(f"ps{i}", [128, PSUM_CAP], f32)) for i in range(8)]

        sW = [ctx.enter_context(nc.semaphore(f"sW{i}")) for i in range(8)]
        # sW indices: 0=e1a lo, 1=e1a hi, 2=e2a, 3=e1b lo, 4=e1b hi, 5=e2b, 6=s1, 7=s2
        sXA = ctx.enter_context(nc.semaphore("sXA"))
        sXB = ctx.enter_context(nc.semaphore("sXB"))
        sXS = ctx.enter_context(nc.semaphore("sXS"))
        sB = ctx.enter_context(nc.semaphore("sB"))
        pe1 = ctx.enter_context(nc.semaphore("pe1"))
        pe2 = ctx.enter_context(nc.semaphore("pe2"))
        act1 = ctx.enter_context(nc.semaphore("act1"))
        dve1 = ctx.enter_context(nc.semaphore("dve1"))
        outS = ctx.enter_context(nc.semaphore("outS"))
        block = ctx.enter_context(nc.Block(no_gpsimd_drain=True))

        E1a, E2a, OTa = E1[:], E2[:], OT[:]

        def w1(s, m):  # stationary for mm1: [128, KC*128] row m
            if s == 2:
                return S1[:][:, m, :]
            return E1a[:, s * KH + m, :]

        def w2(s, m2):  # stationary for mm2
            if s == 2:
                return S2[:][:, m2, :]
            return E2a[:, s * KC + m2, :]

        Xs_ = [XA[:], XB[:], XS[:]]
        H1s_ = [H1A[:], H1B[:], H1S[:]]
        caps = [CA, CB, CS]
        sc1s = [sc1e, sc1e, 1.0]
        sc2s = [sc2e, sc2e, 1.0]
        ooffs = [0, KC * CA, KC * (CA + CB)]

        @block.sync
        def _(sync):
            sync.dma_start(out=E1a[:, 0:KH // 2, :], in_=e1d[0, 0]).then_inc(sW[0], 16)
            sync.dma_start(out=XA[:], in_=xad[:]).then_inc(sXA, 16)
            sync.dma_start(out=E1a[:, KH // 2:KH, :], in_=e1d[0, 1]).then_inc(sW[1], 16)
            sync.dma_start(out=E2a[:, 0:KC, :], in_=e2d[0]).then_inc(sW[2], 16)
            sync.dma_start(out=XB[:], in_=xbd[:]).then_inc(sXB, 16)
            sync.dma_start(out=E1a[:, KH:KH + KH // 2, :], in_=e1d[1, 0]).then_inc(sW[3], 16)
            sync.dma_start(out=E1a[:, KH + KH // 2:2 * KH, :], in_=e1d[1, 1]).then_inc(sW[4], 16)
            sync.dma_start(out=E2a[:, KC:2 * KC, :], in_=e2d[1]).then_inc(sW[5], 16)
            sync.dma_start(out=XS[:], in_=xsd[:]).then_inc(sXS, 16)
            sync.dma_start(out=S1[:], in_=s1d[:]).then_inc(sW[6], 16)
            sync.dma_start(out=S2[:], in_=s2d[:]).then_inc(sW[7], 16)
            # outputs per (segment, m2 group), HWDGE on this ring
            for s in range(3):
                cap, ooff = caps[s], ooffs[s]
                for m2 in range(KC):
                    sync.wait_ge(dve1, 4 * s + m2 + 1)
                    sync.dma_start(
                        out=outd[:, ooff + m2 * cap: ooff + (m2 + 1) * cap],
                        in_=OTa[:, ooff + m2 * cap: ooff + (m2 + 1) * cap],
                    ).then_inc(outS, 16)
            sync.wait_ge(outS, 16 * 12)

        @block.tensor
        def _(tensor):
            xw = [(sXA, (sW[0], sW[1]), sW[2]), (sXB, (sW[3], sW[4]), sW[5]),
                  (sXS, (sW[6], sW[6]), sW[7])]
            for s in range(3):
                cap = caps[s]
                x = Xs_[s]
                h1 = H1s_[s]
                sx, (sw1lo, sw1hi), sw2 = xw[s]
                tensor.wait_ge(sx, 16)
                tensor.wait_ge(sw1lo, 16)
                for m in range(KH):
                    if m == KH // 2:
                        tensor.wait_ge(sw1hi, 16)
                    # psum bank m%4 recycle: previous relu reader done
                    if m >= 4:
                        tensor.wait_ge(act1, 8 * s + (m - 4) + 1)
                    elif s > 0:
                        tensor.wait_ge(act1, 8 * (s - 1) + (m + 4) + 1)
                    wrow = w1(s, m)
                    for k in range(KC):
                        mm = nc.tensor.matmul(
                            PS[m % 4][:, :cap],
                            lhsT=wrow[:, k * 128:(k + 1) * 128],
                            rhs=x[:, k, :],
                            start=(k == 0),
                            stop=(k == KC - 1),
                        )
                    mm.then_inc(pe1, 1)
                tensor.wait_ge(sw2, 16)
                for m2 in range(KC):
                    if s > 0:
                        tensor.wait_ge(dve1, 4 * (s - 1) + m2 + 1)
                    wrow = w2(s, m2)
                    for k2 in range(KH):
                        if m2 == 0:
                            tensor.wait_ge(act1, 8 * s + k2 + 1)
                        mm = nc.tensor.matmul(
                            PS[4 + m2][:, :cap],
                            lhsT=wrow[:, k2 * 128:(k2 + 1) * 128],
                            rhs=h1[:, k2, 0:cap],
                            start=(k2 == 0),
                            stop=(k2 == KH - 1),
                        )
                    mm.then_inc(pe2, 1)

        @block.scalar
        def _(scalar):
            import concourse.mybir as mybir_

            relu = mybir_.ActivationFunctionType.Relu
            scalar.dma_start(out=BI[:], in_=biasd[:]).then_inc(sB, 16)
            scalar.wait_ge(sB, 16)
            # dummy relu: pulls the lazy ACT_TABLE_LOAD off the critical path
            nc.scalar.activation(SC[:][:, 0:1], BI[:][:, 0:1], relu,
                                 bias=BI[:][:, 0:1], scale=1.0)
            for s in range(3):
                cap = caps[s]
                h1 = H1s_[s]
                for m in range(KH):
                    scalar.wait_ge(pe1, 8 * s + m + 1)
                    nc.scalar.activation(
                        h1[:, m, 0:cap],
                        PS[m % 4][:, :cap],
                        relu,
                        bias=BI[:][:, s * BCOLS + m: s * BCOLS + m + 1],
                        scale=float(sc1s[s]),
                    ).then_inc(act1, 1)

        @block.vector
        def _(vector):
            import concourse.mybir as mybir_

            for s in range(3):
                cap, ooff = caps[s], ooffs[s]
                for m2 in range(KC):
                    vector.wait_ge(pe2, 4 * s + m2 + 1)
                    nc.vector.tensor_scalar(
                        OTa[:, ooff + m2 * cap: ooff + (m2 + 1) * cap],
                        PS[4 + m2][:, :cap],
                        float(sc2s[s]),
                        BI[:][:, s * BCOLS + KH + m2: s * BCOLS + KH + m2 + 1],
                        op0=mybir_.AluOpType.mult,
                        op1=mybir_.AluOpType.add,
                    ).then_inc(dve1, 1)

    return nc


def _run_coresim(CA, CB, CS, fp8, sc1e, sc2e, in_maps):
    """Local CoreSim execution (numerics check without hardware)."""
    from types import SimpleNamespace

    from concourse.bass_interp import CoreSim

    results = []
    for c, im in enumerate(in_maps):
        nc = _build_program(CA, CB, CS, fp8, sc1e, sc2e)
        if not nc.is_finalized():
            nc.finalize()
        sim = CoreSim(nc, core_id=0, publish_trace=False)
        for name, val in im.items():
            sim.tensor(name)[:] = val
        sim.simulate()
        results.append({"out": np.array(sim.tensor("out"))})
        print(f"  coresim core {c} done", flush=True)
    return SimpleNamespace(results=results, exec_time_ns=None)


def _route(x2, bucket, expert_key):
    """Host router in float64. Returns gid (N,2), combine weights (N,2)."""
    hn = x2 / np.maximum(np.linalg.norm(x2, axis=-1, keepdims=True), 1e-12)
    keys = expert_key / np.maximum(
        np.linalg.norm(expert_key, axis=-1, keepdims=True), 1e-12
    )
    kb = keys[bucket]  # (N, EPB, C)
    score = np.einsum("nc,nec->ne", hn, kb) / max(TAU, 1e-6)
    score -= score.max(axis=-1, keepdims=True)
    p = np.exp(score)
    p /= p.sum(axis=-1, keepdims=True)
    local = np.argsort(-p, axis=-1, kind="stable")[:, :TOPK]  # (N, 2)
    topv = np.take_along_axis(p, local, axis=-1)
    w = topv / (topv.sum(axis=-1, keepdims=True) + 1e-9)
    gid = bucket[:, None] * EPB + local
    return gid, w


def _pow2floor(v):
    return float(2.0 ** np.floor(np.log2(max(v, 1e-30))))


def _ceil16(n):
    return max(16, -(-int(n) // 16) * 16)


def _wpack1(w1):  # (C,H) -> [128, KH, KC*128] m-major
    return np.ascontiguousarray(
        w1.reshape(KC, 128, KH, 128).transpose(1, 2, 0, 3).reshape(128, KH, KC * 128)
    )


def _wpack2(w2):  # (H,C) -> [128, KC, KH*128] m2-major
    return np.ascontiguousarray(
        w2.reshape(KH, 128, KC, 128).transpose(1, 2, 0, 3).reshape(128, KC, KH * 128)
    )


def kernel(**inputs):
    import ml_dtypes

    _ensure_ntff_hook()
    from concourse.bass_utils import run_bass_kernel_spmd

    bf16 = ml_dtypes.bfloat16
    f8 = ml_dtypes.float8_e4m3

    x = np.asarray(inputs["x"], dtype=np.float32)
    op_id = np.asarray(inputs["op_id"]).astype(np.int64)
    expert_key = np.asarray(inputs["expert_key"], dtype=np.float64)
    sW1 = np.asarray(inputs["sW1"], dtype=np.float32)
    sb1 = np.asarray(inputs["sb1"], dtype=np.float32)
    sW2 = np.asarray(inputs["sW2"], dtype=np.float32)
    sb2 = np.asarray(inputs["sb2"], dtype=np.float32)
    eW1 = np.asarray(inputs["eW1"], dtype=np.float32)
    eb1 = np.asarray(inputs["eb1"], dtype=np.float32)
    eW2 = np.asarray(inputs["eW2"], dtype=np.float32)
    eb2 = np.asarray(inputs["eb2"], dtype=np.float32)
    gate_logit = float(np.asarray(inputs["gate_logit"]))

    B, T, Cc = x.shape
    assert Cc == C
    N = B * T
    assert N % N_CORES == 0
    x2 = x.reshape(N, C)
    bucket = np.clip(op_id.reshape(-1), 0, N_BUCKET - 1)

    gid, w = _route(x2.astype(np.float64), bucket, expert_key)
    gate = 1.0 / (1.0 + np.exp(-gate_logit))

    # ---- assign experts to cores: big+small pairing ---------------------
    flat_gid = gid.reshape(-1)  # slot i -> token i//2
    sorted_slots = np.argsort(flat_gid, kind="stable")
    counts = np.bincount(flat_gid, minlength=E)
    starts = np.concatenate([[0], np.cumsum(counts)])
    assert counts.max() <= PSUM_CAP, "expert overflow; need chunked fallback"
    order = np.argsort(-counts, kind="stable")
    CA = _ceil16(counts[order[0]])
    CB = _ceil16(counts[order[8]])
    CS = N // N_CORES
    TOT = CA + CB + CS
    OC = KC * TOT

    fp8 = gate <= 0.25
    if fp8:
        s_x = _pow2floor(192.0 / max(np.abs(x2).max(), 1e-6))
        s_w1 = _pow2floor(192.0 / max(np.abs(eW1).max(), 1e-6))
        s_w2 = _pow2floor(192.0 / max(np.abs(eW2).max(), 1e-6))
        xn = np.linalg.norm(x2, axis=1).max()
        w1n = np.linalg.norm(eW1, axis=1).max()
        h1_bound = xn * w1n + np.abs(eb1).max() + 1e-6
        s_h = _pow2floor(192.0 / h1_bound)
        sc1e = s_h / (s_x * s_w1)
        sc2e = 1.0 / (s_h * s_w2)
    else:
        s_x = s_w1 = s_w2 = s_h = 1.0
        sc1e, sc2e = 1.0, 1.0
    dt_e = f8 if fp8 else bf16

    key = (CA, CB, CS, fp8, sc1e, sc2e)
    if key not in _BUILD_CACHE:
        _BUILD_CACHE[key] = _build_program(CA, CB, CS, fp8, sc1e, sc2e)
    nc = _BUILD_CACHE[key]

    # ---- host packing ---------------------------------------------------
    x2T = np.ascontiguousarray(x2.T)  # (C, N)
    s1_host = _wpack1(sW1).astype(bf16)
    s2_host = _wpack2(sW2).astype(bf16)

    slot_flat = np.zeros((3, N), np.int64)
    in_maps = []
    for c in range(N_CORES):
        eA, eB = int(order[c]), int(order[15 - c])
        e1h = np.zeros((2, 2, 128, KH // 2, KC * 128), dt_e)
        e2h = np.zeros((2, 128, KC, KH * 128), dt_e)
        for j, e in ((0, eA), (1, eB)):
            w1m = (_wpack1(eW1[e]) * s_w1).astype(dt_e)  # [128, KH, KC*128]
            e1h[j, 0] = w1m[:, :KH // 2]
            e1h[j, 1] = w1m[:, KH // 2:]
            e2h[j] = (_wpack2(eW2[e]) * s_w2).astype(dt_e)

        biash = np.zeros((128, 3 * BCOLS), np.float32)
        xts = [np.zeros((128, KC, CA), dt_e), np.zeros((128, KC, CB), dt_e)]
        for j, (e, capj) in enumerate([(eA, CA), (eB, CB)]):
            toks = (sorted_slots[starts[e]: starts[e + 1]] // TOPK).astype(np.int64)
            n = len(toks)
            xg = x2T[:, toks] * s_x  # (C, n)
            xts[j][:, :, :n] = xg.reshape(KC, 128, n).transpose(1, 0, 2).astype(dt_e)
            biash[:, j * BCOLS: j * BCOLS + KH] = eb1[e].reshape(KH, 128).T * s_h
            biash[:, j * BCOLS + KH: (j + 1) * BCOLS] = eb2[e].reshape(KC, 128).T
            chunk = sorted_slots[starts[e]: starts[e + 1]]
            off = 0 if j == 0 else CA
            slot_flat[chunk % TOPK, toks] = c * TOT + off + np.arange(n)
        stoks = np.arange(c * CS, (c + 1) * CS)
        xsh = np.ascontiguousarray(
            x2T[:, stoks].reshape(KC, 128, CS).transpose(1, 0, 2)
        ).astype(bf16)
        biash[:, 2 * BCOLS: 2 * BCOLS + KH] = sb1.reshape(KH, 128).T
        biash[:, 2 * BCOLS + KH: 3 * BCOLS] = sb2.reshape(KC, 128).T
        slot_flat[2, stoks] = c * TOT + CA + CB + np.arange(CS)

        in_maps.append({
            "e1": e1h, "e2": e2h, "s1": s1_host, "s2": s2_host,
            "xa": xts[0], "xb": xts[1], "xs": xsh, "bias": biash,
        })

    # ---- run on the 8 cores --------------------------------------------
    import os

    global LAST_EXEC_NS, LAST_RESULTS
    if os.environ.get("BASS_SIM"):
        res = _run_coresim(CA, CB, CS, fp8, sc1e, sc2e, in_maps)
    else:
        trace = bool(os.environ.get("BASS_TRACE"))
        res = run_bass_kernel_spmd(
            nc,
            in_maps,
            core_ids=list(range(N_CORES)),
            trace=trace,
            trace_cores=list(range(N_CORES)) if trace else None,
        )
        LAST_EXEC_NS = res.exec_time_ns
        LAST_RESULTS = res

    # ---- un-shard: gather each token's 3 rows and combine ---------------
    allout = np.empty((N_CORES * TOT, C), np.float32)
    caps = [(0, CA), (KC * CA, CB), (KC * (CA + CB), CS)]
    for c in range(N_CORES):
        o = np.asarray(res.results[c]["out"]).astype(np.float32)  # [128, OC]
        row0 = c * TOT
        for ooff, cap in caps:
            blk = o[:, ooff: ooff + KC * cap].reshape(128, KC, cap)
            allout[row0: row0 + cap] = blk.transpose(2, 1, 0).reshape(cap, C)
            row0 += cap

    wf = (gate * w).astype(np.float32)  # (N, 2)
    y = (
        allout[slot_flat[0]] * wf[:, 0:1]
        + allout[slot_flat[1]] * wf[:, 1:2]
        + allout[slot_flat[2]]
    )
    return y.reshape(B, T, C).astype(np.float32)


LAST_EXEC_NS = None
LAST_RESULTS = None


# revision 15
# speedup vs baseline: 1.5320x; 1.0469x over previous
"""MoE FFN with hierarchical KV router — Trainium2 Bass kernel (8 NeuronCores).

Expert-parallel, weights-resident design:
  * Host computes the router (l2-norm scores -> softmax over EPB=4 -> top-2 ->
    combine weights) and dispatches tokens by global expert id.
  * Each core owns TWO experts (big+small pairing over the 16 experts) plus a
    replica of the shared FFN serving 2048/8 = 256 tokens. All weights are
    loaded into SBUF exactly once per core (they stay resident); tokens run
    through three fixed-size segments [CA | CB | CS]:
        seg 0: expert A  (CA token slots)   seg 1: expert B (CB slots)
        seg 2: shared FFN (CS = 256 slots)
    out_seg = relu(x @ W1 + b1) @ W2 + b2   (unweighted; host combines)
  * Precision: bf16 everywhere; when the expert path is strongly attenuated
    (sigmoid(gate_logit) <= 0.25) the expert segments use fp8-e4m3 inputs
    with power-of-2 scaling. Outputs bf16, combined in fp32 on host.
  * Activations travel transposed ([feature, token]) so weights are the
    stationary matmul operand; no on-device transposes.

Schedule notes (from NTFF profiling):
  * HWDGE dma_start costs ~0.7-1us of issue time on the issuing engine, so
    input DMAs are need-ordered on the sync ring (first expert's W1 m<4
    half + its tokens first) and the bias ride the scalar ring.
  * W1 is packed m-major / W2 m2-major so the PE can start after the first
    W1 piece instead of the whole tile.
  * The scalar engine runs a dummy relu right after the bias lands to pull
    the lazy ACT_TABLE_LOAD (~1.5us) off the first real relu.
  * Outputs go out per (segment, m2-group) on the sync HWDGE ring (the
    gpsimd SWDGE path measured ~55 GB/s and added ~4us of tail).
"""
import sys

if "/opt/trn_rl_repo" not in sys.path:
    sys.path.insert(0, "/opt/trn_rl_repo")

import numpy as np

N_BUCKET, EPB, TOPK, TAU = 4, 4, 2, 1.0
C, H = 512, 1024
E = N_BUCKET * EPB
KC, KH = C // 128, H // 128  # contraction blocks: 4, 8
N_CORES = 8
PSUM_CAP = 512
BCOLS = KH + KC  # bias cols per segment

_BUILD_CACHE = {}


def _ensure_ntff_hook():
    """Polyfill antenv.axon_hooks (absent in some agent images) so
    run_bass_kernel_spmd(trace=True) can fetch NTFF profiles."""
    try:
        from antenv.axon_hooks import get_axon_ntff_profile_hook  # noqa: F401
        return
    except ImportError:
        pass
    import types

    try:
        import antenv
        from trn_agent_boot.trn_boot import _ntff_profile_via_ctypes

        hook = _ntff_profile_via_ctypes("/opt/axon/libaxon_pjrt.so")
        mod = types.ModuleType("antenv.axon_hooks")
        state = {"hook": hook}
        mod.get_axon_ntff_profile_hook = lambda: state["hook"]
        mod.set_axon_ntff_profile_hook = lambda h: state.update(hook=h)
        sys.modules["antenv.axon_hooks"] = mod
        antenv.axon_hooks = mod
    except Exception:
        pass


def _build_program(CA, CB, CS, fp8, sc1e, sc2e):
    """One-shot program: 3 segments (expert A, expert B, shared) per core."""
    from contextlib import ExitStack

    import concourse.bass as bass
    import concourse.mybir as mybir

    f32 = mybir.dt.float32
    bf16 = mybir.dt.bfloat16
    dt_e = mybir.dt.float8e4 if fp8 else bf16
    OC = KC * (CA + CB + CS)

    nc = bass.Bass("TRN2", target_bir_lowering=False, debug=False)
    # W1 m-major, expert A in 3 pieces (m01, m23, m47) so the PE can start
    # after the first 0.125MB: e1*[p, m', k*128+q] = W1[k*128+p, m*128+q]
    e1a0 = nc.declare_dram_parameter("e1a0", [128, 2, KC * 128], dt_e, isOutput=False)
    e1a1 = nc.declare_dram_parameter("e1a1", [128, 2, KC * 128], dt_e, isOutput=False)
    e1a2 = nc.declare_dram_parameter("e1a2", [128, KH // 2, KC * 128], dt_e, isOutput=False)
    e1bd = nc.declare_dram_parameter("e1b", [2, 128, KH // 2, KC * 128], dt_e, isOutput=False)
    # W2 m2-major: e2[j, p, m2, k2*128+c'] = W2[k2*128+p, m2*128+c']
    e2d = nc.declare_dram_parameter("e2", [2, 128, KC, KH * 128], dt_e, isOutput=False)
    s1d = nc.declare_dram_parameter("s1", [128, KH, KC * 128], bf16, isOutput=False)
    s2d = nc.declare_dram_parameter("s2", [128, KC, KH * 128], bf16, isOutput=False)
    xad = nc.declare_dram_parameter("xa", [128, KC, CA], dt_e, isOutput=False)
    xbd = nc.declare_dram_parameter("xb", [128, KC, CB], dt_e, isOutput=False)
    xsd = nc.declare_dram_parameter("xs", [128, KC, CS], bf16, isOutput=False)
    biasd = nc.declare_dram_parameter("bias", [128, 3 * BCOLS], f32, isOutput=False)
    outd = nc.declare_dram_parameter("out", [128, OC], bf16, isOutput=True)

    with ExitStack() as ctx:
        E1 = ctx.enter_context(nc.sbuf_tensor("E1", [128, 2 * KH, KC * 128], dt_e))
        E2 = ctx.enter_context(nc.sbuf_tensor("E2", [128, 2 * KC, KH * 128], dt_e))
        S1 = ctx.enter_context(nc.sbuf_tensor("S1", [128, KH, KC * 128], bf16))
        S2 = ctx.enter_context(nc.sbuf_tensor("S2", [128, KC, KH * 128], bf16))
        XA = ctx.enter_context(nc.sbuf_tensor("XA", [128, KC, CA], dt_e))
        XB = ctx.enter_context(nc.sbuf_tensor("XB", [128, KC, CB], dt_e))
        XS = ctx.enter_context(nc.sbuf_tensor("XS", [128, KC, CS], bf16))
        BI = ctx.enter_context(nc.sbuf_tensor("BI", [128, 3 * BCOLS], f32))
        SC = ctx.enter_context(nc.sbuf_tensor("SC", [128, 1], f32))
        DW = ctx.enter_context(nc.sbuf_tensor("DW", [128, 128], dt_e))
        DR = ctx.enter_context(nc.sbuf_tensor("DR", [128, 256], dt_e))
        H1A = ctx.enter_context(nc.sbuf_tensor("H1A", [128, KH, CA], dt_e))
        H1B = ctx.enter_context(nc.sbuf_tensor("H1B", [128, KH, CB], dt_e))
        H1S = ctx.enter_context(nc.sbuf_tensor("H1S", [128, KH, CS], bf16))
        OT = ctx.enter_context(nc.sbuf_tensor("OT", [128, OC], bf16))
        PS = [ctx.enter_context(nc.psum_tensor(f"ps{i}", [128, PSUM_CAP], f32)) for i in range(8)]

        sW = [ctx.enter_context(nc.semaphore(f"sW{i}")) for i in range(9)]
        # sW: 0=e1a m01, 1=e1a m23, 2=e1a m47, 3=e2a, 4=e1b lo, 5=e1b hi,
        #     6=e2b, 7=s1, 8=s2
        dveM = ctx.enter_context(nc.semaphore("dveM"))
        sXA = ctx.enter_context(nc.semaphore("sXA"))
        sXB = ctx.enter_context(nc.semaphore("sXB"))
        sXS = ctx.enter_context(nc.semaphore("sXS"))
        sB = ctx.enter_context(nc.semaphore("sB"))
        pe1 = ctx.enter_context(nc.semaphore("pe1"))
        pe2 = ctx.enter_context(nc.semaphore("pe2"))
        act1 = ctx.enter_context(nc.semaphore("act1"))
        dve1 = ctx.enter_context(nc.semaphore("dve1"))
        outS = ctx.enter_context(nc.semaphore("outS"))
        block = ctx.enter_context(nc.Block(no_gpsimd_drain=True))

        E1a, E2a, OTa = E1[:], E2[:], OT[:]

        def w1(s, m):  # stationary for mm1: [128, KC*128] row m
            if s == 2:
                return S1[:][:, m, :]
            return E1a[:, s * KH + m, :]

        def w2(s, m2):  # stationary for mm2
            if s == 2:
                return S2[:][:, m2, :]
            return E2a[:, s * KC + m2, :]

        Xs_ = [XA[:], XB[:], XS[:]]
        H1s_ = [H1A[:], H1B[:], H1S[:]]
        caps = [CA, CB, CS]
        sc1s = [sc1e, sc1e, 1.0]
        sc2s = [sc2e, sc2e, 1.0]
        ooffs = [0, KC * CA, KC * (CA + CB)]

        @block.sync
        def _(sync):
            sync.dma_start(out=E1a[:, 0:2, :], in_=e1a0[:]).then_inc(sW[0], 16)
            sync.dma_start(out=XA[:], in_=xad[:]).then_inc(sXA, 16)
            sync.dma_start(out=E1a[:, 2:4, :], in_=e1a1[:]).then_inc(sW[1], 16)
            sync.dma_start(out=E1a[:, 4:KH, :], in_=e1a2[:]).then_inc(sW[2], 16)
            sync.dma_start(out=E2a[:, 0:KC, :], in_=e2d[0]).then_inc(sW[3], 16)
            sync.dma_start(out=XB[:], in_=xbd[:]).then_inc(sXB, 16)
            sync.dma_start(out=E1a[:, KH:KH + KH // 2, :], in_=e1bd[0]).then_inc(sW[4], 16)
            sync.dma_start(out=E1a[:, KH + KH // 2:2 * KH, :], in_=e1bd[1]).then_inc(sW[5], 16)
            sync.dma_start(out=E2a[:, KC:2 * KC, :], in_=e2d[1]).then_inc(sW[6], 16)
            sync.dma_start(out=XS[:], in_=xsd[:]).then_inc(sXS, 16)
            sync.dma_start(out=S1[:], in_=s1d[:]).then_inc(sW[7], 16)
            sync.dma_start(out=S2[:], in_=s2d[:]).then_inc(sW[8], 16)
            # outputs: seg0/seg1 in halves, seg2 per m2 group (m2=3 rides
            # the scalar ring so the last two issue in parallel)
            for s in range(2):
                cap, ooff = caps[s], ooffs[s]
                for h in range(2):
                    sync.wait_ge(dve1, 4 * s + 2 * (h + 1))
                    sync.dma_start(
                        out=outd[:, ooff + 2 * h * cap: ooff + 2 * (h + 1) * cap],
                        in_=OTa[:, ooff + 2 * h * cap: ooff + 2 * (h + 1) * cap],
                    ).then_inc(outS, 16)
            cap, ooff = caps[2], ooffs[2]
            for m2 in range(KC - 1):
                sync.wait_ge(dve1, 8 + m2 + 1)
                sync.dma_start(
                    out=outd[:, ooff + m2 * cap: ooff + (m2 + 1) * cap],
                    in_=OTa[:, ooff + m2 * cap: ooff + (m2 + 1) * cap],
                ).then_inc(outS, 16)
            sync.wait_ge(outS, 16 * 8)

        @block.tensor
        def _(tensor):
            # warm up the PE p-state while input DMAs stream in
            tensor.wait_ge(dveM, 2)
            for _ in range(24):
                nc.tensor.matmul(PS[7][:, :256], lhsT=DW[:], rhs=DR[:],
                                 start=True, stop=True)
            xw = [(sXA, (sW[0], sW[1], sW[2]), sW[3]),
                  (sXB, (sW[4], sW[4], sW[5]), sW[6]),
                  (sXS, (sW[7], sW[7], sW[7]), sW[8])]
            for s in range(3):
                cap = caps[s]
                x = Xs_[s]
                h1 = H1s_[s]
                sx, w1waits, sw2 = xw[s]
                tensor.wait_ge(sx, 16)
                tensor.wait_ge(w1waits[0], 16)
                for m in range(KH):
                    if s == 0 and m in (2, 4):
                        tensor.wait_ge(w1waits[m // 2], 16)
                    elif s == 1 and m == KH // 2:
                        tensor.wait_ge(w1waits[2], 16)
                    # psum bank m%4 recycle: previous relu reader done
                    if m >= 4:
                        tensor.wait_ge(act1, 8 * s + (m - 4) + 1)
                    elif s > 0:
                        tensor.wait_ge(act1, 8 * (s - 1) + (m + 4) + 1)
                    wrow = w1(s, m)
                    for k in range(KC):
                        mm = nc.tensor.matmul(
                            PS[m % 4][:, :cap],
                            lhsT=wrow[:, k * 128:(k + 1) * 128],
                            rhs=x[:, k, :],
                            start=(k == 0),
                            stop=(k == KC - 1),
                        )
                    mm.then_inc(pe1, 1)
                tensor.wait_ge(sw2, 16)
                for m2 in range(KC):
                    if s > 0:
                        tensor.wait_ge(dve1, 4 * (s - 1) + m2 + 1)
                    wrow = w2(s, m2)
                    for k2 in range(KH):
                        if m2 == 0:
                            tensor.wait_ge(act1, 8 * s + k2 + 1)
                        mm = nc.tensor.matmul(
                            PS[4 + m2][:, :cap],
                            lhsT=wrow[:, k2 * 128:(k2 + 1) * 128],
                            rhs=h1[:, k2, 0:cap],
                            start=(k2 == 0),
                            stop=(k2 == KH - 1),
                        )
                    mm.then_inc(pe2, 1)

        @block.scalar
        def _(scalar):
            import concourse.mybir as mybir_

            relu = mybir_.ActivationFunctionType.Relu
            scalar.dma_start(out=BI[:], in_=biasd[:]).then_inc(sB, 16)
            scalar.wait_ge(sB, 16)
            # dummy relu: pulls the lazy ACT_TABLE_LOAD off the critical path
            nc.scalar.activation(SC[:][:, 0:1], BI[:][:, 0:1], relu,
                                 bias=BI[:][:, 0:1], scale=1.0)
            for s in range(3):
                cap = caps[s]
                h1 = H1s_[s]
                for m in range(KH):
                    scalar.wait_ge(pe1, 8 * s + m + 1)
                    nc.scalar.activation(
                        h1[:, m, 0:cap],
                        PS[m % 4][:, :cap],
                        relu,
                        bias=BI[:][:, s * BCOLS + m: s * BCOLS + m + 1],
                        scale=float(sc1s[s]),
                    ).then_inc(act1, 1)
            # final output piece on this ring, in parallel with sync's
            cap, ooff = caps[2], ooffs[2]
            scalar.wait_ge(dve1, 12)
            scalar.dma_start(
                out=outd[:, ooff + 3 * cap: ooff + 4 * cap],
                in_=OTa[:, ooff + 3 * cap: ooff + 4 * cap],
            ).then_inc(outS, 16)

        @block.vector
        def _(vector):
            import concourse.mybir as mybir_

            nc.vector.memset(DW[:], 0.0).then_inc(dveM, 1)
            nc.vector.memset(DR[:], 0.0).then_inc(dveM, 1)
            for s in range(3):
                cap, ooff = caps[s], ooffs[s]
                for m2 in range(KC):
                    vector.wait_ge(pe2, 4 * s + m2 + 1)
                    nc.vector.tensor_scalar(
                        OTa[:, ooff + m2 * cap: ooff + (m2 + 1) * cap],
                        PS[4 + m2][:, :cap],
                        float(sc2s[s]),
                        BI[:][:, s * BCOLS + KH + m2: s * BCOLS + KH + m2 + 1],
                        op0=mybir_.AluOpType.mult,
                        op1=mybir_.AluOpType.add,
                    ).then_inc(dve1, 1)

    return nc


def _run_coresim(CA, CB, CS, fp8, sc1e, sc2e, in_maps):
    """Local CoreSim execution (numerics check without hardware)."""
    from types import SimpleNamespace

    from concourse.bass_interp import CoreSim

    results = []
    for c, im in enumerate(in_maps):
        nc = _build_program(CA, CB, CS, fp8, sc1e, sc2e)
        if not nc.is_finalized():
            nc.finalize()
        sim = CoreSim(nc, core_id=0, publish_trace=False)
        for name, val in im.items():
            sim.tensor(name)[:] = val
        sim.simulate()
        results.append({"out": np.array(sim.tensor("out"))})
        print(f"  coresim core {c} done", flush=True)
    return SimpleNamespace(results=results, exec_time_ns=None)


def _route(x2, bucket, expert_key):
    """Host router in float64. Returns gid (N,2), combine weights (N,2)."""
    hn = x2 / np.maximum(np.linalg.norm(x2, axis=-1, keepdims=True), 1e-12)
    keys = expert_key / np.maximum(
        np.linalg.norm(expert_key, axis=-1, keepdims=True), 1e-12
    )
    kb = keys[bucket]  # (N, EPB, C)
    score = np.einsum("nc,nec->ne", hn, kb) / max(TAU, 1e-6)
    score -= score.max(axis=-1, keepdims=True)
    p = np.exp(score)
    p /= p.sum(axis=-1, keepdims=True)
    local = np.argsort(-p, axis=-1, kind="stable")[:, :TOPK]  # (N, 2)
    topv = np.take_along_axis(p, local, axis=-1)
    w = topv / (topv.sum(axis=-1, keepdims=True) + 1e-9)
    gid = bucket[:, None] * EPB + local
    return gid, w


def _pow2floor(v):
    return float(2.0 ** np.floor(np.log2(max(v, 1e-30))))


def _ceil16(n):
    return max(16, -(-int(n) // 16) * 16)


def _wpack1(w1):  # (C,H) -> [128, KH, KC*128] m-major
    return np.ascontiguousarray(
        w1.reshape(KC, 128, KH, 128).transpose(1, 2, 0, 3).reshape(128, KH, KC * 128)
    )


def _wpack2(w2):  # (H,C) -> [128, KC, KH*128] m2-major
    return np.ascontiguousarray(
        w2.reshape(KH, 128, KC, 128).transpose(1, 2, 0, 3).reshape(128, KC, KH * 128)
    )


def kernel(**inputs):
    import ml_dtypes

    _ensure_ntff_hook()
    from concourse.bass_utils import run_bass_kernel_spmd

    bf16 = ml_dtypes.bfloat16
    f8 = ml_dtypes.float8_e4m3

    x = np.asarray(inputs["x"], dtype=np.float32)
    op_id = np.asarray(inputs["op_id"]).astype(np.int64)
    expert_key = np.asarray(inputs["expert_key"], dtype=np.float64)
    sW1 = np.asarray(inputs["sW1"], dtype=np.float32)
    sb1 = np.asarray(inputs["sb1"], dtype=np.float32)
    sW2 = np.asarray(inputs["sW2"], dtype=np.float32)
    sb2 = np.asarray(inputs["sb2"], dtype=np.float32)
    eW1 = np.asarray(inputs["eW1"], dtype=np.float32)
    eb1 = np.asarray(inputs["eb1"], dtype=np.float32)
    eW2 = np.asarray(inputs["eW2"], dtype=np.float32)
    eb2 = np.asarray(inputs["eb2"], dtype=np.float32)
    gate_logit = float(np.asarray(inputs["gate_logit"]))

    B, T, Cc = x.shape
    assert Cc == C
    N = B * T
    assert N % N_CORES == 0
    x2 = x.reshape(N, C)
    bucket = np.clip(op_id.reshape(-1), 0, N_BUCKET - 1)

    gid, w = _route(x2.astype(np.float64), bucket, expert_key)
    gate = 1.0 / (1.0 + np.exp(-gate_logit))

    # ---- assign experts to cores: big+small pairing ---------------------
    flat_gid = gid.reshape(-1)  # slot i -> token i//2
    sorted_slots = np.argsort(flat_gid, kind="stable")
    counts = np.bincount(flat_gid, minlength=E)
    starts = np.concatenate([[0], np.cumsum(counts)])
    assert counts.max() <= PSUM_CAP, "expert overflow; need chunked fallback"
    order = np.argsort(-counts, kind="stable")
    CA = _ceil16(counts[order[0]])
    CB = _ceil16(counts[order[8]])
    CS = N // N_CORES
    TOT = CA + CB + CS
    OC = KC * TOT

    fp8 = gate <= 0.25
    if fp8:
        s_x = _pow2floor(192.0 / max(np.abs(x2).max(), 1e-6))
        s_w1 = _pow2floor(192.0 / max(np.abs(eW1).max(), 1e-6))
        s_w2 = _pow2floor(192.0 / max(np.abs(eW2).max(), 1e-6))
        xn = np.linalg.norm(x2, axis=1).max()
        w1n = np.linalg.norm(eW1, axis=1).max()
        h1_bound = xn * w1n + np.abs(eb1).max() + 1e-6
        s_h = _pow2floor(192.0 / h1_bound)
        sc1e = s_h / (s_x * s_w1)
        sc2e = 1.0 / (s_h * s_w2)
    else:
        s_x = s_w1 = s_w2 = s_h = 1.0
        sc1e, sc2e = 1.0, 1.0
    dt_e = f8 if fp8 else bf16

    key = (CA, CB, CS, fp8, sc1e, sc2e)
    if key not in _BUILD_CACHE:
        _BUILD_CACHE[key] = _build_program(CA, CB, CS, fp8, sc1e, sc2e)
    nc = _BUILD_CACHE[key]

    # ---- host packing ---------------------------------------------------
    x2T = np.ascontiguousarray(x2.T)  # (C, N)
    s1_host = _wpack1(sW1).astype(bf16)
    s2_host = _wpack2(sW2).astype(bf16)

    slot_flat = np.zeros((3, N), np.int64)
    in_maps = []
    for c in range(N_CORES):
        eA, eB = int(order[c]), int(order[15 - c])
        e2h = np.zeros((2, 128, KC, KH * 128), dt_e)
        w1a = (_wpack1(eW1[eA]) * s_w1).astype(dt_e)  # [128, KH, KC*128]
        w1b = (_wpack1(eW1[eB]) * s_w1).astype(dt_e)
        e1bh = np.stack([w1b[:, :KH // 2], w1b[:, KH // 2:]])
        e2h[0] = (_wpack2(eW2[eA]) * s_w2).astype(dt_e)
        e2h[1] = (_wpack2(eW2[eB]) * s_w2).astype(dt_e)

        biash = np.zeros((128, 3 * BCOLS), np.float32)
        xts = [np.zeros((128, KC, CA), dt_e), np.zeros((128, KC, CB), dt_e)]
        for j, (e, capj) in enumerate([(eA, CA), (eB, CB)]):
            toks = (sorted_slots[starts[e]: starts[e + 1]] // TOPK).astype(np.int64)
            n = len(toks)
            xg = x2T[:, toks] * s_x  # (C, n)
            xts[j][:, :, :n] = xg.reshape(KC, 128, n).transpose(1, 0, 2).astype(dt_e)
            biash[:, j * BCOLS: j * BCOLS + KH] = eb1[e].reshape(KH, 128).T * s_h
            biash[:, j * BCOLS + KH: (j + 1) * BCOLS] = eb2[e].reshape(KC, 128).T
            chunk = sorted_slots[starts[e]: starts[e + 1]]
            off = 0 if j == 0 else CA
            slot_flat[chunk % TOPK, toks] = c * TOT + off + np.arange(n)
        stoks = np.arange(c * CS, (c + 1) * CS)
        xsh = np.ascontiguousarray(
            x2T[:, stoks].reshape(KC, 128, CS).transpose(1, 0, 2)
        ).astype(bf16)
        biash[:, 2 * BCOLS: 2 * BCOLS + KH] = sb1.reshape(KH, 128).T
        biash[:, 2 * BCOLS + KH: 3 * BCOLS] = sb2.reshape(KC, 128).T
        slot_flat[2, stoks] = c * TOT + CA + CB + np.arange(CS)

        in_maps.append({
            "e1a0": np.ascontiguousarray(w1a[:, 0:2]),
            "e1a1": np.ascontiguousarray(w1a[:, 2:4]),
            "e1a2": np.ascontiguousarray(w1a[:, 4:KH]),
            "e1b": e1bh, "e2": e2h, "s1": s1_host, "s2": s2_host,
            "xa": xts[0], "xb": xts[1], "xs": xsh, "bias": biash,
        })

    # ---- run on the 8 cores --------------------------------------------
    import os

    global LAST_EXEC_NS, LAST_RESULTS
    if os.environ.get("BASS_SIM"):
        res = _run_coresim(CA, CB, CS, fp8, sc1e, sc2e, in_maps)
    else:
        trace = bool(os.environ.get("BASS_TRACE"))
        res = run_bass_kernel_spmd(
            nc,
            in_maps,
            core_ids=list(range(N_CORES)),
            trace=trace,
            trace_cores=list(range(N_CORES)) if trace else None,
        )
        LAST_EXEC_NS = res.exec_time_ns
        LAST_RESULTS = res

    # ---- un-shard: gather each token's 3 rows and combine ---------------
    allout = np.empty((N_CORES * TOT, C), np.float32)
    caps = [(0, CA), (KC * CA, CB), (KC * (CA + CB), CS)]
    for c in range(N_CORES):
        o = np.asarray(res.results[c]["out"]).astype(np.float32)  # [128, OC]
        row0 = c * TOT
        for ooff, cap in caps:
            blk = o[:, ooff: ooff + KC * cap].reshape(128, KC, cap)
            allout[row0: row0 + cap] = blk.transpose(2, 1, 0).reshape(cap, C)
            row0 += cap

    wf = (gate * w).astype(np.float32)  # (N, 2)
    y = (
        allout[slot_flat[0]] * wf[:, 0:1]
        + allout[slot_flat[1]] * wf[:, 1:2]
        + allout[slot_flat[2]]
    )
    return y.reshape(B, T, C).astype(np.float32)


LAST_EXEC_NS = None
LAST_RESULTS = None
